# revision 1
# baseline (speedup 1.0000x reference)
"""Trainium2 Bass kernel for the MFPA attention module.

Reference computation (per batch b, with N = H*W = 4096 spatial sites):
    q = Wq @ x_RGB + bq            (CQK=16 channels)
    k = Wk @ x    + bk
    v = Wv @ x    + bv             (C=64 channels)
    energy[i,j] = q_i . k_j
    att = softmax(energy, axis=j)
    out[c,i] = sum_j v[c,j] att[i,j]
    y = lam * out + x

Device strategy (8 NeuronCores): data-parallel over batch (4) x query-row
halves (2).  Each core holds x[b] fully (for K/V and the residual) and its
2048-row query slice, and computes a flash-style streaming softmax so the
4096x4096 energy matrix never leaves PSUM/SBUF.

Host-side weight folding (softmax is shift-invariant, so bk drops out):
    energy[i,j] = (M^T xr_i + bqk) . xf_j    with  M = Wq^T Wk, bqk = Wk^T bq
V is computed on-device as xf_aug^T @ wv_aug where xf_aug carries a ones row
and wv_aug carries bv as its last row plus a ones column that makes the PV
matmul also produce the softmax row-sums for free.
"""

import ml_dtypes
import numpy as np

import concourse.bass as bass
import concourse.mybir as mybir
import concourse.tile as tile_mod
from concourse.vector_clock import ScopedClock

B, C, HH, WW = 4, 64, 64, 64
N = HH * WW          # 4096 spatial sites
NI = N // 2          # query rows per core
CHUNK = 512          # query rows processed per main-loop iteration
NCHUNK = NI // CHUNK
JBLK = 128           # key/value block (PSUM partition dim)
NJ = N // JBLK       # 32 j-blocks
JGRP = 2             # j-blocks per PSUM/exp group
NCORES = 8

F32 = mybir.dt.float32
F32R = mybir.dt.float32r
BF16 = mybir.dt.bfloat16


def _patched_drain_and_barrier(self, tick_clock, wait_clock):
    # The walrus build in this container rejects instructions with more than
    # one sync-wait command ("Too many sync wait commands" on the Tile tail
    # drain).  Split the aggregated drain into one drain per semaphore wait.
    nc = self.nc
    drain_inst = nc.sync.drain()
    wait_clock.add_sem_waits(
        drain_inst.ins, ScopedClock({None: tick_clock.global_clock})
    )
    inst = drain_inst.ins
    si = inst.sync_info
    waits = list(si.on_wait or []) if si else []
    if len(waits) > 1:
        si.on_wait = waits[:1]
        for w in waits[1:]:
            extra = nc.sync.drain()
            extra.ins.sync_info = mybir.SyncInfo(on_wait=[w], on_update=[])
    nc.all_engine_barrier()
    popped = nc._tile_sem_poison_stack.pop()
    assert popped is self._sem_poison
    nc.clear_and_free_semaphores(list(self.sems.allocated().values()))
    nc.all_engine_barrier()


tile_mod.TileContext._drain_and_barrier = _patched_drain_and_barrier


def _split_multi_waits(nc):
    # This walrus build accepts at most one sync-wait command per TPB
    # instruction.  Hoist extra waits onto engine NoOps placed just before
    # the instruction (engine executes in order, so semantics are kept).
    for blk in nc.m.functions[0].blocks:
        insts = list(blk.instructions)
        out = []
        changed = False
        for inst in insts:
            si = inst.sync_info
            if si is not None and si.on_wait and len(si.on_wait) > 1:
                waits = list(si.on_wait)
                si.on_wait = waits[-1:]
                for w in waits[:-1]:
                    nop = mybir.InstNoOp(name=nc.get_next_instruction_name())
                    nop.engine = inst.engine
                    nop.sync_info = mybir.SyncInfo(on_wait=[w], on_update=[])
                    out.append(nop)
                changed = True
            out.append(inst)
        if changed:
            blk.instructions = out


def build_bass(split_waits=True):
    nc = bass.Bass()
    xf = nc.declare_dram_parameter("xf", [C + 1, N], BF16, isOutput=False)
    xres = nc.declare_dram_parameter("xres", [C, NI], F32, isOutput=False)
    xq = nc.declare_dram_parameter("xq", [C, NI], BF16, isOutput=False)
    m = nc.declare_dram_parameter("m", [C, C], BF16, isOutput=False)
    bqk = nc.declare_dram_parameter("bqk", [C, 1], F32, isOutput=False)
    wv = nc.declare_dram_parameter("wv", [C + 1, C + 2], BF16, isOutput=False)
    onesv = nc.declare_dram_parameter("onesv", [1, C], F32R, isOutput=False)
    y = nc.declare_dram_parameter("y", [C, NI], F32, isOutput=True)

    with tile_mod.TileContext(nc) as tc:
        with (
            tc.tile_pool(name="singles", bufs=1) as singles,
            tc.tile_pool(name="qkpool", bufs=4) as qkpool,
            tc.tile_pool(name="ppool", bufs=4) as ppool,
            tc.tile_pool(name="ypool", bufs=3) as ypool,
            tc.tile_pool(name="small", bufs=4) as small,
            tc.tile_pool(name="ps_prep", bufs=1, space="PSUM") as ps_prep,
            tc.tile_pool(name="ps_qk", bufs=1, space="PSUM") as ps_qk,
            tc.tile_pool(name="ps_lrb", bufs=1, space="PSUM") as ps_lrb,
            tc.tile_pool(name="ps_et", bufs=2, space="PSUM") as ps_et,
            tc.tile_pool(name="ps_pv", bufs=1, space="PSUM") as ps_pv,
        ):
            # ---- load constants and inputs -------------------------------
            # chunked input DMAs so early consumers start as soon as their
            # slice lands instead of waiting for the whole tensor
            xf_sb = singles.tile([C + 1, N], BF16)
            for k in range(8):
                ks = slice(k * (N // 8), (k + 1) * (N // 8))
                nc.sync.dma_start(out=xf_sb[:, ks], in_=xf[:, ks])
            xq_sb = singles.tile([C, NI], BF16)
            for k in range(NCHUNK):
                ks = slice(k * CHUNK, (k + 1) * CHUNK)
                nc.gpsimd.dma_start(out=xq_sb[:, ks], in_=xq[:, ks])
            xres_sb = singles.tile([C, NI], F32)
            for k in range(NCHUNK):
                ks = slice(k * CHUNK, (k + 1) * CHUNK)
                nc.gpsimd.dma_start(out=xres_sb[:, ks], in_=xres[:, ks])
            m_sb = singles.tile([C, C], BF16)
            nc.gpsimd.dma_start(out=m_sb, in_=m[:, :])
            bqk_sb = singles.tile([C, 1], F32)
            nc.gpsimd.dma_start(out=bqk_sb, in_=bqk[:, :])
            wv_sb = singles.tile([C + 1, C + 2], BF16)
            nc.gpsimd.dma_start(out=wv_sb, in_=wv[:, :])
            ones_sb = singles.tile([1, C], F32R)
            nc.gpsimd.dma_start(out=ones_sb, in_=onesv[:, :])
            # xf arrives column-permuted so this core's own query half sits
            # at columns 0:NI — softmax is invariant under j-permutation, and
            # it makes the residual slice xf_sb[:, 0:NI] a static AP.

            # ---- Q.K preparation for all chunks (fills the xf-DMA wait) --
            qk_sbs = []
            for ic in range(NCHUNK):
                isl = slice(ic * CHUNK, (ic + 1) * CHUNK)
                qs = ps_qk.tile([C, CHUNK], F32)
                nc.tensor.matmul(
                    out=qs, lhsT=m_sb, rhs=xq_sb[:, isl], start=True, stop=True
                )
                qk_sb = qkpool.tile([C, CHUNK], BF16)
                nc.vector.tensor_scalar_add(qk_sb, qs, bqk_sb)
                qk_sbs.append(qk_sb)

            # ---- V preparation: v_aug[j, c] in (j, c) layout -------------
            v_sb = singles.tile([JBLK, NJ, C + 1], BF16)
            for jb in range(NJ):
                vp = ps_prep.tile([JBLK, C + 2], F32)
                nc.tensor.matmul(
                    out=vp,
                    lhsT=xf_sb[:, jb * JBLK : (jb + 1) * JBLK],
                    rhs=wv_sb,
                    start=True,
                    stop=True,
                )
                nc.vector.tensor_copy(v_sb[:, jb, :], vp[:, 0 : C + 1])

            # ---- main loop over query chunks -----------------------------
            def epilogue(pv, ic):
                isl = slice(ic * CHUNK, (ic + 1) * CHUNK)

                r_t = small.tile([1, CHUNK], F32R)
                with nc.allow_low_precision(reason="f32r copy of softmax recip"):
                    nc.vector.reciprocal(out=r_t, in_=pv[C : C + 1, :])
                # broadcast 1/s_i across partitions via outer product
                lrb = ps_lrb.tile([C, CHUNK], F32)
                nc.tensor.matmul(
                    out=lrb, lhsT=ones_sb, rhs=r_t, start=True, stop=True
                )
                lrb_sb = small.tile([C, CHUNK], F32)
                nc.vector.tensor_copy(lrb_sb, lrb)
                y_t = ypool.tile([C, CHUNK], F32)
                nc.vector.tensor_tensor(
                    out=y_t, in0=pv[0:C, :], in1=lrb_sb, op=mybir.AluOpType.mult
                )
                nc.vector.tensor_tensor(
                    out=y_t, in0=y_t, in1=xres_sb[:, isl], op=mybir.AluOpType.add
                )
                nc.sync.dma_start(out=y[:, isl], in_=y_t)

            for ic in range(NCHUNK):
                qk_sb = qk_sbs[ic]
                pv = ps_pv.tile([C + 1, CHUNK], F32)
                for jg in range(NJ // JGRP):
                    et = ps_et.tile([JBLK, JGRP, CHUNK], F32)
                    for g in range(JGRP):
                        jb = jg * JGRP + g
                        nc.tensor.matmul(
                            out=et[:, g, :],
                            lhsT=xf_sb[0:C, jb * JBLK : (jb + 1) * JBLK],
                            rhs=qk_sb,
                            start=True,
                            stop=True,
                        )
                    p_t = ppool.tile([JBLK, JGRP, CHUNK], BF16)
                    nc.scalar.activation(
                        out=p_t, in_=et, func=mybir.ActivationFunctionType.Exp
                    )
                    for g in range(JGRP):
                        jb = jg * JGRP + g
                        nc.tensor.matmul(
                            out=pv,
                            lhsT=v_sb[:, jb, :],
                            rhs=p_t[:, g, :],
                            start=(jb == 0),
                            stop=(jb == NJ - 1),
                        )

                epilogue(pv, ic)

    if split_waits:
        _split_multi_waits(nc)
    return nc


_CACHE = {}


def kernel(**inputs):
    x = np.ascontiguousarray(np.asarray(inputs["x"], dtype=np.float32))
    x_RGB = np.ascontiguousarray(np.asarray(inputs["x_RGB"], dtype=np.float32))
    Wq = np.asarray(inputs["Wq"], dtype=np.float32)
    bq = np.asarray(inputs["bq"], dtype=np.float32)
    Wk = np.asarray(inputs["Wk"], dtype=np.float32)
    Wv = np.asarray(inputs["Wv"], dtype=np.float32)
    bv = np.asarray(inputs["bv"], dtype=np.float32)
    lam = np.asarray(inputs["lam"], dtype=np.float32)

    M = (Wq.T.astype(np.float64) @ Wk.astype(np.float64)).astype(np.float32)
    bqk = (Wk.T.astype(np.float64) @ bq.astype(np.float64)).astype(np.float32)

    wv_aug = np.zeros((C + 1, C + 2), np.float32)
    wv_aug[:C, :C] = Wv.T
    wv_aug[C, :C] = bv
    wv_aug[:, :C] *= float(lam.reshape(-1)[0])
    wv_aug[C, C] = 1.0

    xf3 = x.reshape(B, C, N)
    xr3 = x_RGB.reshape(B, C, N)

    if "nc" not in _CACHE:
        _CACHE["nc"] = build_bass()
    nc = _CACHE["nc"]

    in_maps = []
    for core in range(NCORES):
        b, ih = core >> 1, core & 1
        xf_aug = np.empty((C + 1, N), np.float32)
        # own query half first (static residual slice), other half after
        xf_aug[:C, :NI] = xf3[b][:, ih * NI : (ih + 1) * NI]
        xf_aug[:C, NI:] = xf3[b][:, (1 - ih) * NI : (2 - ih) * NI]
        xf_aug[C] = 1.0
        in_maps.append(
            {
                "xf": xf_aug.astype(ml_dtypes.bfloat16),
                "xres": np.ascontiguousarray(xf_aug[:C, :NI]),
                "xq": np.ascontiguousarray(
                    xr3[b][:, ih * NI : (ih + 1) * NI]
                ).astype(ml_dtypes.bfloat16),
                "m": M.astype(ml_dtypes.bfloat16),
                "bqk": bqk.reshape(C, 1),
                "wv": wv_aug.astype(ml_dtypes.bfloat16),
                "onesv": np.ones((1, C), np.float32),
            }
        )

    from concourse.bass_utils import run_bass_kernel_spmd

    res = run_bass_kernel_spmd(nc, in_maps, list(range(NCORES)))

    out = np.empty((B, C, N), np.float32)
    for core in range(NCORES):
        b, ih = core >> 1, core & 1
        out[b][:, ih * NI : (ih + 1) * NI] = res.results[core]["y"]
    return out.reshape(B, C, HH, WW)

